# revision 30
# baseline (speedup 1.0000x reference)
"""Trainium2 Bass kernel for nn_DenseConv2d_full (dense_cnn).

Computation per sample b (8 samples, data-parallel over 8 NeuronCores):
  step 1: x[(ci,cr), y, w] = sum_{dy<16, dx<8} resp[cr,dy,dx] * imp[ci, y-dy, w-dx]
          (causal / top-left-cropped full conv)
  step 2: y[oc, y, w] = bias[oc] + sum_{(ci,cr), my, mx} conv_w[oc,(ci,cr),my,mx]
          * x[(ci,cr), y+my-1, w+mx-1]   (3x3 SAME conv)

Device mapping (all matmul operands bf16, accumulation f32 in PSUM):
  Phase A (step 1): per (ci, 16-row block): 2 accumulating matmuls
    [K=128 = (4 dx-shifts x 32 impulse rows), M=128=(16 y)x(8 cr), N=512]
    with Toeplitz-banded response weights. The 4 dx-shifted copies of the
    impulse rows are pre-materialized on the host and loaded one 4MB DMA
    per block; per block one 2MB store stages x to a DRAM scratch in
    [blk][(yl,cr)][ci][w] bf16 layout (per-dma_start cost ~1us dominates
    on this platform, so DMAs are batched as large as the 3-dim access
    pattern limit allows).
  Phase B (step 2): per output row pair (Y, Y+1): 9 tap matmuls each
    [K=128 ch, M=64 oc, N=512] on the two 128x64 PE column tiles
    (T0 -> psum partitions 0-63 = row Y, T1 -> 64-127 = row Y+1),
    ScalarE bias-add evacuates PSUM directly, one [128, 512] DMA stores
    both rows. Phases run sequentially (all A, then all B): interleaving
    them measured slower on HW (PE tiling-mode switch drains).
"""

import os
import sys
from contextlib import ExitStack

import numpy as np

for _p in (
    "/root/.axon_site",
    "/root/.axon_site/_ro/trn_rl_repo",
    "/root/.axon_site/_ro/pypackages",
    "/opt/trn_rl_repo",
):
    if os.path.isdir(_p) and _p not in sys.path:
        sys.path.append(_p)

import concourse.bass as bass  # noqa: E402
import concourse.tile as tile  # noqa: E402
from concourse import bacc, mybir  # noqa: E402
from concourse.bass_utils import run_bass_kernel_spmd  # noqa: E402

F32 = mybir.dt.float32
BF16 = mybir.dt.bfloat16

B, CR, KH, KW = 8, 8, 16, 8
CI, H, W = 16, 256, 512
OC = 64
NBLK = H // 16  # 16 blocks of 16 output rows

_BUILT = {}
# Sequential phases measure ~1.7x faster than block-interleaved on HW:
# interleaving the 128x128 phase-A matmuls with phase-B's 128x64 column-
# tiled matmuls forces PE tiling-mode drains and psum-pool contention.
INTERLEAVE = False


def _row_src(x_scr, y):
    """DRAM AP [ci:16, cr:8, x:W]: element = x_scr[blk, 8*yl+cr, ci, x].

    x_scr layout is [blk][(yl,cr) partition][ci][x]; this gathers one x row
    into the (ci*8+cr)-partition order phase B contracts over.
    """
    blk, yl = divmod(y, 16)
    base = x_scr[:]
    off = blk * 128 * CI * W + yl * 8 * CI * W
    return bass.AP(
        tensor=base.tensor,
        offset=base.offset + off,
        ap=[[W, CI], [CI * W, 8], [1, W]],
    )


def _sbuf_2col_dst(t, width, inner, count):
    """SBUF AP [128, count, inner] into tile t ([128, width]) at offset 1."""
    return bass.AP(
        tensor=t.tensor,
        offset=t.offset + 1,
        ap=[[width, 128], [inner + 2, count], [1, inner]],
    )


def _build_nc(epochs=1, phases="AB", bench_mode=False):
    nc = bacc.Bacc(
        "TRN2",
        target_bir_lowering=False,
        debug=False,
        enable_asserts=False,
        num_devices=8,
    )
    ikind = "Internal" if bench_mode else "ExternalInput"
    imp_big = nc.dram_tensor(
        "imp_big", [NBLK, 128, CI, 2, W], BF16, kind=ikind
    ).ap()
    w_toe = nc.dram_tensor("w_toe", [128, 2, 128], BF16, kind=ikind).ap()
    w9 = nc.dram_tensor("w9", [128, 9, OC], BF16, kind=ikind).ap()
    bias2 = nc.dram_tensor("bias2", [128, 1], F32, kind=ikind).ap()
    if bench_mode:
        nc.dram_tensor("dummy_in", [1, 1], F32, kind="ExternalInput")
    y_out = nc.dram_tensor(
        "y_out", [OC, H, W], F32, kind="Internal" if bench_mode else "ExternalOutput"
    ).ap()
    done = (
        nc.dram_tensor("done", [128, 1], F32, kind="ExternalOutput").ap()
        if bench_mode
        else None
    )
    x_scr = nc.dram_tensor("x_scr", [NBLK, 128, CI, W], BF16).ap()

    with tile.TileContext(nc) as tc, ExitStack() as ctx:
        consts = ctx.enter_context(tc.tile_pool(name="consts", bufs=1))
        imp_pool = ctx.enter_context(tc.tile_pool(name="imp", bufs=2))
        xev_pool = ctx.enter_context(tc.tile_pool(name="xev", bufs=2))
        rows_pool = ctx.enter_context(tc.tile_pool(name="rows", bufs=10))
        yout_pool = ctx.enter_context(tc.tile_pool(name="yt", bufs=8))
        psum_pool = ctx.enter_context(tc.tile_pool(name="psum", bufs=8, space="PSUM"))

        wt = consts.tile([128, 2, 128], BF16)
        nc.sync.dma_start(wt[:], w_toe[:])
        w9t = consts.tile([128, 9, OC], BF16)
        nc.sync.dma_start(w9t[:], w9[:])
        bt = consts.tile([128, 1], F32)
        nc.sync.dma_start(bt[:], bias2[:])
        zrow = consts.tile([128, W + 2], BF16)
        nc.vector.memset(zrow[:], 0.0)

        for _ep in range(epochs):
            _phase_ab(nc, tc, locals(), phases)
        if done is not None:
            nc.sync.dma_start(done, bt[:])

    nc.compile()
    return nc


def _phase_ab(nc, tc, env, phases="AB"):
    imp_big, y_out, x_scr = env["imp_big"], env["y_out"], env["x_scr"]
    imp_pool, xev_pool, rows_pool, yout_pool, psum_pool = (
        env["imp_pool"],
        env["xev_pool"],
        env["rows_pool"],
        env["yout_pool"],
        env["psum_pool"],
    )
    wt, w9t, bt, zrow, _ep = env["wt"], env["w9t"], env["bt"], env["zrow"], env["_ep"]

    def a_block(blk):
        # one 4MB load: it_big[(s,u), ci*1024 + g*512 + x] (host-prearranged)
        it = imp_pool.tile([128, CI * 2 * W], BF16, tag="imp")
        nc.sync.dma_start(it[:], imp_big[blk])
        xb = xev_pool.tile([128, CI * W], BF16, tag="xe")
        for ci in range(CI):
            ps = psum_pool.tile([128, W], F32, tag="ps", name=f"psA_{_ep}_{blk}_{ci}")
            for g in range(2):
                nc.tensor.matmul(
                    ps[:, :],
                    lhsT=wt[:, 1 - g, :],
                    rhs=it[:, ci * 2 * W + g * W : ci * 2 * W + (g + 1) * W],
                    start=(g == 0),
                    stop=(g == 1),
                )
            nc.vector.tensor_copy(xb[:, ci * W : (ci + 1) * W], ps[:])
        # one 2MB store: x_scr[blk] is laid out exactly like xb
        nc.scalar.dma_start(x_scr[blk], xb.rearrange("p (ci x) -> p ci x", ci=CI))

    # ---------------- Phase B ----------------
    # row-pair tiles: tile q holds x rows (2q, 2q+1) at col offsets 1 and W+3
    WP = 2 * (W + 2)
    rowt = {}

    def get_pair(q):
        if q in rowt:
            return rowt[q]
        t = rows_pool.tile([128, WP], BF16, tag="row")
        # zero guard cols {0, W+1, W+2, 2W+3} in one strided memset
        nc.vector.memset(
            bass.AP(
                tensor=t.tensor,
                offset=t.offset,
                ap=[[WP, 128], [W + 2, 2], [W + 1, 2]],
            ),
            0.0,
        )
        for j, eng in ((0, nc.sync), (1, nc.scalar)):
            eng.dma_start(
                t[:, 1 + j * (W + 2) : 1 + j * (W + 2) + W],
                _row_src(x_scr, 2 * q + j),
            )
        rowt[q] = t
        for qq in list(rowt):
            if qq < q - 1:
                del rowt[qq]
        return t

    def row_ref(y):
        # -> (tile, base col) for x row y; zrow for out-of-range
        if y < 0 or y >= H:
            return zrow, 1
        q, j = divmod(y, 2)
        return get_pair(q), 1 + j * (W + 2)

    def b_pair(Y):
        # rows Y (psum partitions 0-63 via column tile T0) and Y+1 (64-127, T1)
        ps = psum_pool.tile([128, W], F32, tag="ps", name=f"psB_{_ep}_{Y}")
        rts = [row_ref(Y - 1), row_ref(Y), row_ref(Y + 1), row_ref(Y + 2)]
        for half in range(2):
            for t9 in range(9):
                my, mx = divmod(t9, 3)
                t, base = rts[my + half]
                nc.tensor.matmul(
                    ps[64 * half : 64 * half + 64, :],
                    lhsT=w9t[:, t9, :],
                    rhs=t[:, base + mx - 1 : base + mx - 1 + W],
                    start=(t9 == 0),
                    stop=(t9 == 8),
                    tile_position=(0, 64 * half),
                    skip_group_check=True,
                )
        # bias-add + PSUM evacuation on ScalarE, per half (partition-aligned)
        y2 = yout_pool.tile([128, W], F32, tag="y2")
        nc.scalar.add(y2[0:64, :], ps[0:64, :], bt[0:64, :])
        nc.scalar.add(y2[64:128, :], ps[64:128, :], bt[64:128, :])
        (nc.sync if (Y // 2) % 2 else nc.scalar).dma_start(
            y_out[:, Y : Y + 2, :].rearrange("oc h w -> h oc w"),
            y2[:],
        )

    # ---- driver: interleaved (A(blk) overlaps B of block blk-2) or
    # sequential (all A then all B; avoids PE tiling-mode switches and
    # psum-pool contention between the phases) ----
    if "A" in phases and "B" in phases:
        if INTERLEAVE:
            for blk in range(NBLK):
                a_block(blk)
                if blk >= 2:
                    for q in range(8 * (blk - 2), 8 * (blk - 1)):
                        b_pair(2 * q)
            for bb in (NBLK - 2, NBLK - 1):
                for q in range(8 * bb, 8 * bb + 8):
                    b_pair(2 * q)
        else:
            for blk in range(NBLK):
                a_block(blk)
            for q in range(H // 2):
                b_pair(2 * q)
    elif "A" in phases:
        for blk in range(NBLK):
            a_block(blk)
    elif "B" in phases:
        for q in range(H // 2):
            b_pair(2 * q)


def _host_prep(response, impulse, conv_w, conv_b):
    """Per-sample input prep (pure layout + bf16 cast, no flops)."""
    import ml_dtypes

    bf16 = ml_dtypes.bfloat16
    in_maps = []
    # w9[(ci*8+cr), my*3+mx, oc] = conv_w[oc, ci*8+cr, my, mx]
    w9 = np.ascontiguousarray(conv_w.transpose(1, 2, 3, 0).reshape(128, 9, OC)).astype(
        bf16
    )
    bias2 = np.tile(conv_b.astype(np.float32), 2).reshape(128, 1)
    # w_toe[(s,u), dxg, yl*8+cr] = resp[cr, yl+16-u, dxg*4+3-s]
    # vectorized: dy[u, yl] = yl + 16 - u  (valid when 0 <= dy < 16)
    u_idx = np.arange(32)[:, None]
    yl_idx = np.arange(16)[None, :]
    dy = yl_idx + 16 - u_idx  # [32, 16]
    valid = (dy >= 0) & (dy < KH)
    for b in range(B):
        imp1 = np.zeros((CI, H + 16, W + 8), bf16)
        imp1[:, 16:, 8:] = impulse[b].astype(bf16)
        # imp_big[blk, (s,u), ci, g, x] = imp1[ci, 16*blk+u, 1+4*g+s+x]
        imp_big = np.empty((NBLK, 4, 32, CI, 2, W), bf16)
        for s in range(4):
            for g in range(2):
                c0 = 1 + 4 * g + s
                sl = imp1[:, :, c0 : c0 + W]  # [CI, 272, W]
                blocks = np.lib.stride_tricks.sliding_window_view(
                    sl, 32, axis=1
                )[:, ::16]  # [CI, 16, W, 32]
                imp_big[:, s, :, :, g, :] = blocks.transpose(1, 3, 0, 2)
        imp_big = imp_big.reshape(NBLK, 128, CI, 2, W)
        wt1 = np.zeros((4, 32, 2, 16, 8), np.float32)  # [s, u, dxg, yl, cr]
        resp = response[b]  # [cr, dy, dx]
        for s in range(4):
            for dxg in range(2):
                dx = dxg * 4 + 3 - s
                # [u, yl, cr] = resp[cr, dy[u,yl], dx] where valid
                r = resp[:, np.clip(dy, 0, KH - 1), dx]  # [cr, 32, 16]
                wt1[s, :, dxg] = np.where(
                    valid[None], r, 0.0
                ).transpose(1, 2, 0)
        in_maps.append(
            {
                "imp_big": imp_big,
                "w_toe": wt1.reshape(128, 2, 128).astype(bf16),
                "w9": w9,
                "bias2": bias2,
            }
        )
    return in_maps


def kernel(response, impulse, conv_w, conv_b, _trace=False):
    response = np.asarray(response, np.float32)
    impulse = np.asarray(impulse, np.float32)
    conv_w = np.asarray(conv_w, np.float32)
    conv_b = np.asarray(conv_b, np.float32)

    if "nc" not in _BUILT:
        _BUILT["nc"] = _build_nc()
    nc = _BUILT["nc"]

    in_maps = _host_prep(response, impulse, conv_w, conv_b)
    res = run_bass_kernel_spmd(nc, in_maps, list(range(B)), trace=_trace)
    out = np.stack([res.results[b]["y_out"] for b in range(B)], axis=0)
    if _trace:
        _BUILT["last_exec_time_ns"] = res.exec_time_ns
        _BUILT["last_results"] = res
    return out


if __name__ == "__main__":
    data = np.load(os.path.join(os.path.dirname(__file__), "ref_cache.npz"))
    out = kernel(data["response"], data["impulse"], data["conv_w"], data["conv_b"])
    ref = data["out"]
    err = np.abs(out - ref).max() / np.abs(ref).max()
    print("Relative error:", err)


# revision 31
# speedup vs baseline: 1.0961x; 1.0961x over previous
"""Trainium2 Bass kernel for nn_DenseConv2d_full (dense_cnn).

Computation per sample b (8 samples, data-parallel over 8 NeuronCores):
  step 1: x[(ci,cr), y, w] = sum_{dy<16, dx<8} resp[cr,dy,dx] * imp[ci, y-dy, w-dx]
          (causal / top-left-cropped full conv)
  step 2: y[oc, y, w] = bias[oc] + sum_{(ci,cr), my, mx} conv_w[oc,(ci,cr),my,mx]
          * x[(ci,cr), y+my-1, w+mx-1]   (3x3 SAME conv)

Device mapping (all matmul operands bf16, accumulation f32 in PSUM):
  Phase A (step 1): per (ci, 16-row block): 2 accumulating matmuls
    [K=128 = (4 dx-shifts x 32 impulse rows), M=128=(16 y)x(8 cr), N=512]
    with Toeplitz-banded response weights. The 4 dx-shifted copies of the
    impulse rows are pre-materialized on the host and loaded one 4MB DMA
    per block; per block one 2MB store stages x to a DRAM scratch in
    [blk][(yl,cr)][ci][w] bf16 layout (per-dma_start cost ~1us dominates
    on this platform, so DMAs are batched as large as the 3-dim access
    pattern limit allows).
  Phase B (step 2): per output row pair (Y, Y+1): 9 tap matmuls each
    [K=128 ch, M=64 oc, N=512] on the two 128x64 PE column tiles
    (T0 -> psum partitions 0-63 = row Y, T1 -> 64-127 = row Y+1),
    ScalarE bias-add evacuates PSUM directly, one [128, 512] DMA stores
    both rows. Phases run sequentially (all A, then all B): interleaving
    them measured slower on HW (PE tiling-mode switch drains).
"""

import os
import sys
from contextlib import ExitStack

import numpy as np

for _p in (
    "/root/.axon_site",
    "/root/.axon_site/_ro/trn_rl_repo",
    "/root/.axon_site/_ro/pypackages",
    "/opt/trn_rl_repo",
):
    if os.path.isdir(_p) and _p not in sys.path:
        sys.path.append(_p)

import concourse.bass as bass  # noqa: E402
import concourse.tile as tile  # noqa: E402
from concourse import bacc, mybir  # noqa: E402
from concourse.bass_utils import run_bass_kernel_spmd  # noqa: E402

F32 = mybir.dt.float32
BF16 = mybir.dt.bfloat16

B, CR, KH, KW = 8, 8, 16, 8
CI, H, W = 16, 256, 512
OC = 64
NBLK = H // 16  # 16 blocks of 16 output rows

_BUILT = {}
# Sequential phases measure ~1.7x faster than block-interleaved on HW:
# interleaving the 128x128 phase-A matmuls with phase-B's 128x64 column-
# tiled matmuls forces PE tiling-mode drains and psum-pool contention.
INTERLEAVE = False


def _row_src(x_scr, y):
    """DRAM AP [ci:16, cr:8, x:W]: element = x_scr[blk, 8*yl+cr, ci, x].

    x_scr layout is [blk][(yl,cr) partition][ci][x]; this gathers one x row
    into the (ci*8+cr)-partition order phase B contracts over.
    """
    blk, yl = divmod(y, 16)
    base = x_scr[:]
    off = blk * 128 * CI * W + yl * 8 * CI * W
    return bass.AP(
        tensor=base.tensor,
        offset=base.offset + off,
        ap=[[W, CI], [CI * W, 8], [1, W]],
    )


def _sbuf_2col_dst(t, width, inner, count):
    """SBUF AP [128, count, inner] into tile t ([128, width]) at offset 1."""
    return bass.AP(
        tensor=t.tensor,
        offset=t.offset + 1,
        ap=[[width, 128], [inner + 2, count], [1, inner]],
    )


def _build_nc(epochs=1, phases="AB", bench_mode=False):
    nc = bacc.Bacc(
        "TRN2",
        target_bir_lowering=False,
        debug=False,
        enable_asserts=False,
        num_devices=8,
    )
    ikind = "Internal" if bench_mode else "ExternalInput"
    imp_big = nc.dram_tensor(
        "imp_big", [NBLK, 128, CI, 2, W], BF16, kind=ikind
    ).ap()
    w_toe = nc.dram_tensor("w_toe", [128, 2, 128], BF16, kind=ikind).ap()
    w9 = nc.dram_tensor("w9", [128, 9, OC], BF16, kind=ikind).ap()
    bias2 = nc.dram_tensor("bias2", [128, 1], F32, kind=ikind).ap()
    if bench_mode:
        nc.dram_tensor("dummy_in", [1, 1], F32, kind="ExternalInput")
    y_out = nc.dram_tensor(
        "y_out", [OC, H, W], F32, kind="Internal" if bench_mode else "ExternalOutput"
    ).ap()
    done = (
        nc.dram_tensor("done", [128, 1], F32, kind="ExternalOutput").ap()
        if bench_mode
        else None
    )
    x_scr = nc.dram_tensor("x_scr", [NBLK, 128, CI, W], BF16).ap()

    with tile.TileContext(nc) as tc, ExitStack() as ctx:
        consts = ctx.enter_context(tc.tile_pool(name="consts", bufs=1))
        imp_pool = ctx.enter_context(tc.tile_pool(name="imp", bufs=3))
        xev_pool = ctx.enter_context(tc.tile_pool(name="xev", bufs=2))
        rows_pool = ctx.enter_context(tc.tile_pool(name="rows", bufs=10))
        yout_pool = ctx.enter_context(tc.tile_pool(name="yt", bufs=8))
        psum_pool = ctx.enter_context(tc.tile_pool(name="psum", bufs=8, space="PSUM"))

        wt = consts.tile([128, 2, 128], BF16)
        nc.sync.dma_start(wt[:], w_toe[:])
        w9t = consts.tile([128, 9, OC], BF16)
        nc.sync.dma_start(w9t[:], w9[:])
        bt = consts.tile([128, 1], F32)
        nc.sync.dma_start(bt[:], bias2[:])
        zrow = consts.tile([128, W + 2], BF16)
        nc.vector.memset(zrow[:], 0.0)

        for _ep in range(epochs):
            _phase_ab(nc, tc, locals(), phases)
        if done is not None:
            nc.sync.dma_start(done, bt[:])

    nc.compile()
    return nc


def _phase_ab(nc, tc, env, phases="AB"):
    imp_big, y_out, x_scr = env["imp_big"], env["y_out"], env["x_scr"]
    imp_pool, xev_pool, rows_pool, yout_pool, psum_pool = (
        env["imp_pool"],
        env["xev_pool"],
        env["rows_pool"],
        env["yout_pool"],
        env["psum_pool"],
    )
    wt, w9t, bt, zrow, _ep = env["wt"], env["w9t"], env["bt"], env["zrow"], env["_ep"]

    def a_block(blk):
        # one 4MB load: it_big[(s,u), ci*1024 + g*512 + x] (host-prearranged);
        # alternate blocks across the two HWDGE queues to balance them
        it = imp_pool.tile([128, CI * 2 * W], BF16, tag="imp")
        (nc.sync if blk % 2 else nc.scalar).dma_start(it[:], imp_big[blk])
        xb = xev_pool.tile([128, CI * W], BF16, tag="xe")
        for ci in range(CI):
            ps = psum_pool.tile([128, W], F32, tag="ps", name=f"psA_{_ep}_{blk}_{ci}")
            for g in range(2):
                nc.tensor.matmul(
                    ps[:, :],
                    lhsT=wt[:, 1 - g, :],
                    rhs=it[:, ci * 2 * W + g * W : ci * 2 * W + (g + 1) * W],
                    start=(g == 0),
                    stop=(g == 1),
                )
            nc.vector.tensor_copy(xb[:, ci * W : (ci + 1) * W], ps[:])
        # one 2MB store: x_scr[blk] is laid out exactly like xb
        (nc.scalar if blk % 2 else nc.sync).dma_start(
            x_scr[blk], xb.rearrange("p (ci x) -> p ci x", ci=CI)
        )

    # ---------------- Phase B ----------------
    # row-pair tiles: tile q holds x rows (2q, 2q+1) at col offsets 1 and W+3
    WP = 2 * (W + 2)
    rowt = {}

    def get_pair(q):
        if q in rowt:
            return rowt[q]
        t = rows_pool.tile([128, WP], BF16, tag="row")
        # zero guard cols {0, W+1, W+2, 2W+3} in one strided memset
        nc.vector.memset(
            bass.AP(
                tensor=t.tensor,
                offset=t.offset,
                ap=[[WP, 128], [W + 2, 2], [W + 1, 2]],
            ),
            0.0,
        )
        for j, eng in ((0, nc.sync), (1, nc.scalar)):
            eng.dma_start(
                t[:, 1 + j * (W + 2) : 1 + j * (W + 2) + W],
                _row_src(x_scr, 2 * q + j),
            )
        rowt[q] = t
        for qq in list(rowt):
            if qq < q - 1:
                del rowt[qq]
        return t

    def row_ref(y):
        # -> (tile, base col) for x row y; zrow for out-of-range
        if y < 0 or y >= H:
            return zrow, 1
        q, j = divmod(y, 2)
        return get_pair(q), 1 + j * (W + 2)

    def b_pair(Y):
        # rows Y (psum partitions 0-63 via column tile T0) and Y+1 (64-127, T1)
        ps = psum_pool.tile([128, W], F32, tag="ps", name=f"psB_{_ep}_{Y}")
        rts = [row_ref(Y - 1), row_ref(Y), row_ref(Y + 1), row_ref(Y + 2)]
        for half in range(2):
            for t9 in range(9):
                my, mx = divmod(t9, 3)
                t, base = rts[my + half]
                nc.tensor.matmul(
                    ps[64 * half : 64 * half + 64, :],
                    lhsT=w9t[:, t9, :],
                    rhs=t[:, base + mx - 1 : base + mx - 1 + W],
                    start=(t9 == 0),
                    stop=(t9 == 8),
                    tile_position=(0, 64 * half),
                    skip_group_check=True,
                )
        # bias-add + PSUM evacuation on ScalarE, per half (partition-aligned)
        y2 = yout_pool.tile([128, W], F32, tag="y2")
        nc.scalar.add(y2[0:64, :], ps[0:64, :], bt[0:64, :])
        nc.scalar.add(y2[64:128, :], ps[64:128, :], bt[64:128, :])
        (nc.sync if (Y // 2) % 2 else nc.scalar).dma_start(
            y_out[:, Y : Y + 2, :].rearrange("oc h w -> h oc w"),
            y2[:],
        )

    # ---- driver: interleaved (A(blk) overlaps B of block blk-2) or
    # sequential (all A then all B; avoids PE tiling-mode switches and
    # psum-pool contention between the phases) ----
    if "A" in phases and "B" in phases:
        if INTERLEAVE:
            for blk in range(NBLK):
                a_block(blk)
                if blk >= 2:
                    for q in range(8 * (blk - 2), 8 * (blk - 1)):
                        b_pair(2 * q)
            for bb in (NBLK - 2, NBLK - 1):
                for q in range(8 * bb, 8 * bb + 8):
                    b_pair(2 * q)
        else:
            for blk in range(NBLK):
                a_block(blk)
            for q in range(H // 2):
                b_pair(2 * q)
    elif "A" in phases:
        for blk in range(NBLK):
            a_block(blk)
    elif "B" in phases:
        for q in range(H // 2):
            b_pair(2 * q)


def _host_prep(response, impulse, conv_w, conv_b):
    """Per-sample input prep (pure layout + bf16 cast, no flops)."""
    import ml_dtypes

    bf16 = ml_dtypes.bfloat16
    in_maps = []
    # w9[(ci*8+cr), my*3+mx, oc] = conv_w[oc, ci*8+cr, my, mx]
    w9 = np.ascontiguousarray(conv_w.transpose(1, 2, 3, 0).reshape(128, 9, OC)).astype(
        bf16
    )
    bias2 = np.tile(conv_b.astype(np.float32), 2).reshape(128, 1)
    # w_toe[(s,u), dxg, yl*8+cr] = resp[cr, yl+16-u, dxg*4+3-s]
    # vectorized: dy[u, yl] = yl + 16 - u  (valid when 0 <= dy < 16)
    u_idx = np.arange(32)[:, None]
    yl_idx = np.arange(16)[None, :]
    dy = yl_idx + 16 - u_idx  # [32, 16]
    valid = (dy >= 0) & (dy < KH)
    for b in range(B):
        imp1 = np.zeros((CI, H + 16, W + 8), bf16)
        imp1[:, 16:, 8:] = impulse[b].astype(bf16)
        # imp_big[blk, (s,u), ci, g, x] = imp1[ci, 16*blk+u, 1+4*g+s+x]
        imp_big = np.empty((NBLK, 4, 32, CI, 2, W), bf16)
        for s in range(4):
            for g in range(2):
                c0 = 1 + 4 * g + s
                sl = imp1[:, :, c0 : c0 + W]  # [CI, 272, W]
                blocks = np.lib.stride_tricks.sliding_window_view(
                    sl, 32, axis=1
                )[:, ::16]  # [CI, 16, W, 32]
                imp_big[:, s, :, :, g, :] = blocks.transpose(1, 3, 0, 2)
        imp_big = imp_big.reshape(NBLK, 128, CI, 2, W)
        wt1 = np.zeros((4, 32, 2, 16, 8), np.float32)  # [s, u, dxg, yl, cr]
        resp = response[b]  # [cr, dy, dx]
        for s in range(4):
            for dxg in range(2):
                dx = dxg * 4 + 3 - s
                # [u, yl, cr] = resp[cr, dy[u,yl], dx] where valid
                r = resp[:, np.clip(dy, 0, KH - 1), dx]  # [cr, 32, 16]
                wt1[s, :, dxg] = np.where(
                    valid[None], r, 0.0
                ).transpose(1, 2, 0)
        in_maps.append(
            {
                "imp_big": imp_big,
                "w_toe": wt1.reshape(128, 2, 128).astype(bf16),
                "w9": w9,
                "bias2": bias2,
            }
        )
    return in_maps


def kernel(response, impulse, conv_w, conv_b, _trace=False):
    response = np.asarray(response, np.float32)
    impulse = np.asarray(impulse, np.float32)
    conv_w = np.asarray(conv_w, np.float32)
    conv_b = np.asarray(conv_b, np.float32)

    if "nc" not in _BUILT:
        _BUILT["nc"] = _build_nc()
    nc = _BUILT["nc"]

    in_maps = _host_prep(response, impulse, conv_w, conv_b)
    res = run_bass_kernel_spmd(nc, in_maps, list(range(B)), trace=_trace)
    out = np.stack([res.results[b]["y_out"] for b in range(B)], axis=0)
    if _trace:
        _BUILT["last_exec_time_ns"] = res.exec_time_ns
        _BUILT["last_results"] = res
    return out


if __name__ == "__main__":
    data = np.load(os.path.join(os.path.dirname(__file__), "ref_cache.npz"))
    out = kernel(data["response"], data["impulse"], data["conv_w"], data["conv_b"])
    ref = data["out"]
    err = np.abs(out - ref).max() / np.abs(ref).max()
    print("Relative error:", err)
